# revision 1
# baseline (speedup 1.0000x reference)
"""Multi-head self-attention (RoPE, causal) on 8 Trainium2 NeuronCores.

Problem: B=1, S=2048, D=1024, H=16 heads, d_k=64, causal, interleaved RoPE.

Sharding: tensor-parallel over heads for QKV+attention (2 heads/core),
then AllToAll to switch to sequence sharding, so the output projection
is fully local (each core computes y rows [256c:256c+256] for all 1024
output dims). Host concatenates row slices — no host arithmetic.

Device layouts (per core c, local heads h0=2c, h1=2c+1):
  xt   [1024,2048]  x^T (d on partitions), fp16 — replicated
  qt/kt [128,2048]  Q^T/K^T rows: [h0-even(32) h0-odd(32) h1-even h1-odd]
                    (RoPE pair-permutation folded into weight slices;
                     scores are invariant to a shared permutation of d_k)
  v_sb [128,16*66]  V' tiles [V(64) | ones | pad] per k-tile (ones col ->
                    softmax sums ride the AV matmul as output row 64; the
                    pad col keeps every fp16 ones DMA write 4B-aligned —
                    stride 65 corrupted neighboring V elements on HW)
  scores S^T [k,q] in PSUM; causal mask = accumulating matmul that adds
  -1e9 to the upper-triangle of diagonal 128x128 blocks (I.T @ mtri);
  exp on ACT -> fp16 P^T tiles -> AV: attn^T = V'^T P
  normalize: rec=1/sums (DVE, direct from PSUM), gpsimd
  partition_broadcast, DVE multiply (PSUM src) -> fp16 attnT
  A2A (fp16) shards attn^T [128e, 256s] -> each core gets attnT[:, S_c]
  WO: y[s,m] accumulated over 8 e-tiles from SBUF-resident WO^T (fp16).

All matmuls run fp16 inputs (10-bit mantissa ~ f32r precision) with fp32 PSUM accumulation. The QKV
projections for s-chunk j+1 are emitted interleaved with the attention
pairs of chunk j so the PE stays busy while ACT chews the exp stream.
"""

import math
import numpy as np

import concourse.bass as bass
import concourse.mybir as mybir
import concourse.tile as tile
from concourse import bacc
from concourse.bass_utils import run_bass_kernel_spmd

F32 = mybir.dt.float32
F16 = mybir.dt.float16
AF = mybir.ActivationFunctionType
ALU = mybir.AluOpType

S = 2048
D = 1024
H = 16
DK = 64
NCORES = 8
EC = D // NCORES          # 128 e-dims per core (2 heads)
SC = S // NCORES          # 256 s-rows per core after A2A
NQ = 512                  # q-chunk width
NJ = S // NQ              # 4 q-chunks
KT = S // 128             # 16 k-tiles
DT = D // 128             # 8 d-tiles
THETA = 10000.0

_PROGRAM = None

_HINTS = (mybir.EngineType.PE, mybir.EngineType.Activation,
          mybir.EngineType.DVE, mybir.EngineType.Pool,
          mybir.EngineType.SP)


def _build_program(reps=1, collective=True, loop_stages=("single",),
                   stages=None, look=3, ptbufs=6, debug_taps=False,
                   mask_mode="pe", interleave=True, norm_mode="psum"):
    nc = bacc.Bacc("TRN2", target_bir_lowering=False, debug=False,
                   num_devices=NCORES if collective else 1)

    # ---- DRAM I/O ----
    # weights arrive host-prepacked in their SBUF layouts so each is one
    # wide-line DMA
    xt_d = nc.dram_tensor("xt", [D, S], F16, kind="ExternalInput").ap()
    wqt_d = nc.dram_tensor("wqt", [128, DT * EC], F16,
                           kind="ExternalInput").ap()
    wkt_d = nc.dram_tensor("wkt", [128, DT * EC], F16,
                           kind="ExternalInput").ap()
    wvt_d = nc.dram_tensor("wvt", [128, DT * EC], F16,
                           kind="ExternalInput").ap()
    wot_d = nc.dram_tensor("wot", [128, DT * D], F16,
                           kind="ExternalInput").ap()
    ctab_d = nc.dram_tensor("ctab", [128, S], F16, kind="ExternalInput").ap()
    stab_d = nc.dram_tensor("stab", [128, S], F16, kind="ExternalInput").ap()
    pswap_d = nc.dram_tensor("pswap", [128, 128], F16,
                             kind="ExternalInput").ap()
    mtri_d = nc.dram_tensor("mtri", [128, 128], F16,
                            kind="ExternalInput").ap()
    ident_d = nc.dram_tensor("ident", [128, 128], F16,
                             kind="ExternalInput").ap()
    msk_d = nc.dram_tensor("msk01", [128, 128], F16,
                           kind="ExternalInput").ap()
    ones_d = nc.dram_tensor("ones", [128, KT], F16, kind="ExternalInput").ap()
    y_d = nc.dram_tensor("y_out", [SC, D], F32, kind="ExternalOutput").ap()

    # internal DRAM for the collective
    a2a_in = nc.dram_tensor("a2a_in", [NCORES, EC, SC], F16)
    a2a_out = nc.dram_tensor("a2a_out", [NCORES, EC, SC], F16)

    scale = 1.0 / math.sqrt(DK)

    with tile.TileContext(nc) as tc:
        with (
            tc.tile_pool(name="persist", bufs=1) as pp,
            tc.tile_pool(name="work", bufs=3) as wp,
            tc.tile_pool(name="pt_pool", bufs=ptbufs) as ptp,
            tc.tile_pool(name="psum", bufs=2, space="PSUM") as ps,
            tc.tile_pool(name="psum_att", bufs=2, space="PSUM") as psa,
        ):
            # ---- resident tiles ----
            wqt = pp.tile([128, DT * EC], F16)   # [d-tile part, t*EC+e]
            wkt = pp.tile([128, DT * EC], F16)
            wvt = pp.tile([128, DT * EC], F16)
            pswap = pp.tile([128, 128], F16)
            mtri = pp.tile([128, 128], F16)
            ident = pp.tile([128, 128], F16)
            msk01 = pp.tile([128, 128], F16)
            ctab = pp.tile([128, S], F16)
            stab = pp.tile([128, S], F16)
            xt = [pp.tile([128, S], F16, name=f"xt{t}") for t in range(DT)]
            wot_sb = pp.tile([128, DT * D], F16)  # [e-tile part, t*1024+m]

            def trig_load(jc):
                csl_ = slice(NQ * jc, NQ * (jc + 1))
                nc.sync.dma_start(ctab[:, csl_], ctab_d[:, csl_])
                nc.sync.dma_start(stab[:, csl_], stab_d[:, csl_])

            def xt_load(jc):
                csl_ = slice(NQ * jc, NQ * (jc + 1))
                for t in range(DT):
                    nc.sync.dma_start(
                        xt[t][:, csl_], xt_d[128 * t:128 * (t + 1), csl_])

            def wot_load(q):
                sl = slice(2 * D * q, 2 * D * (q + 1))
                nc.sync.dma_start(wot_sb[:, sl], wot_d[:, sl])

            def head_loads():
                nc.sync.dma_start(pswap[:], pswap_d[:])
                nc.sync.dma_start(mtri[:], mtri_d[:])
                nc.sync.dma_start(ident[:], ident_d[:])
                nc.sync.dma_start(msk01[:], msk_d[:])
                nc.sync.dma_start(wqt[:], wqt_d[:])
                trig_load(0)
                xt_load(0)
                nc.sync.dma_start(wkt[:], wkt_d[:])
                nc.sync.dma_start(wvt[:], wvt_d[:])
                trig_load(1)
                xt_load(1)

            qt = pp.tile([128, S], F16)   # RoPE'd Q^T
            kt = pp.tile([128, S], F16)   # RoPE'd K^T
            v_sb = [pp.tile([128, KT * 66], F16, name=f"v{h}")
                    for h in range(2)]
            attnT = [pp.tile([64, S], F16, name=f"attnT{h}")
                     for h in range(2)]
            at = [pp.tile([128, SC], F16, name=f"at{t}",
                          tag=f"xt{t}" if reps == 1 else f"at{t}")
                  for t in range(DT)]

            def ones_load():
                for h in range(2):
                    nc.sync.dma_start(v_sb[h][:, 64::66], ones_d[:])

            def proj_rope(wt, out_sb, j):
                # q/k projection for s-chunk j + interleaved-pair RoPE
                csl = slice(NQ * j, NQ * (j + 1))
                g_ps = ps.tile([128, NQ], F32, name="g_ps", tag="mm")
                for t in range(DT):
                    nc.tensor.matmul(
                        g_ps[:],
                        wt[:, EC * t:EC * (t + 1)],
                        xt[t][:, csl],
                        start=(t == 0), stop=(t == DT - 1))
                graw = wp.tile([128, NQ], F16, name="graw")
                nc.scalar.copy(graw[:], g_ps[:])
                gsw_ps = ps.tile([128, NQ], F32, name="gsw_ps", tag="mm")
                nc.tensor.matmul(gsw_ps[:], pswap[:], graw[:],
                                 start=True, stop=True)
                a_sb = wp.tile([128, NQ], F16, name="a_sb")
                nc.gpsimd.tensor_mul(a_sb[:], graw[:], ctab[:, csl])
                b_sb = wp.tile([128, NQ], F16, name="b_sb")
                nc.vector.tensor_mul(b_sb[:], gsw_ps[:], stab[:, csl])
                nc.vector.tensor_add(out_sb[:, csl], a_sb[:], b_sb[:])

            def v_proj(st):
                # V tile for s-tile st, [s,e] layout, both heads
                # (full-width PSUM tile: the "mm" tag rotation requires
                # uniform shapes)
                v_ps = ps.tile([128, NQ], F32, name="v_ps", tag="mm")
                for t in range(DT):
                    nc.tensor.matmul(
                        v_ps[:, 0:128],
                        xt[t][:, 128 * st:128 * (st + 1)],
                        wvt[:, EC * t:EC * (t + 1)],
                        start=(t == 0), stop=(t == DT - 1))
                for h in range(2):
                    nc.vector.tensor_copy(
                        v_sb[h][:, 66 * st:66 * st + 64],
                        v_ps[:, 64 * h:64 * (h + 1)])

            def qkv_tokens(j):
                toks = [lambda: proj_rope(wqt, qt, j),
                        lambda: proj_rope(wkt, kt, j)]
                for st in range(4 * j, 4 * (j + 1)):
                    toks.append(lambda st=st: v_proj(st))
                return toks

            def att_chunk(j, filler=(), pre=()):
                # scores + exp + AV for q-chunk j; `pre` thunks (DMA issues
                # for later chunks) go first, then `filler` thunks (QKV for
                # chunk j+1) are emitted between pairs to keep PE busy while
                # ACT runs the exp stream.
                for thunk in pre:
                    thunk()
                ndiag = 4 * j
                npairs = (ndiag + 4) // 2
                pairs = [(p, h) for p in range(npairs) for h in range(2)]
                pend = {}
                av_ps = [psa.tile([65, NQ], F32, name=f"av_ps{h}",
                                  tag="av", bufs=2) for h in range(2)]

                def emit_score(k):
                    p, h = pairs[k]
                    i0 = 2 * p
                    rs = [i0 - ndiag, i0 + 1 - ndiag]
                    offs = [128 * r if r > 0 else 0 for r in rs]
                    ws = [NQ - o for o in offs]
                    cs = [0, ws[0]]
                    wtot = ws[0] + ws[1]
                    hs = slice(64 * h, 64 * (h + 1))
                    st2 = psa.tile([128, 2 * NQ], F32, name=f"st2{h}",
                                   tag="st2", bufs=2)
                    for q in range(2):
                        nc.tensor.matmul(
                            st2[:, cs[q]:cs[q] + ws[q]],
                            kt[hs, 128 * (i0 + q):128 * (i0 + q + 1)],
                            qt[hs, NQ * j + offs[q]:NQ * (j + 1)],
                            start=True, stop=True,
                            tile_position=(64 * h, 0))
                        if rs[q] >= 0 and mask_mode == "pe":
                            # causal mask: add -6e4 to upper triangle of the
                            # diagonal 128-col block via accumulating matmul
                            nc.tensor.matmul(
                                st2[:, cs[q]:cs[q] + 128],
                                ident[:], mtri[:],
                                start=False, stop=True,
                                skip_group_check=True)
                    pt = ptp.tile([128, 2 * NQ], F16, name="pt")
                    nc.scalar.activation(pt[:, :wtot], st2[:, :wtot],
                                         AF.Exp, scale=scale)
                    if rs[0] >= 0 and mask_mode == "dve":
                        for q in range(2):
                            nc.vector.tensor_mul(
                                pt[:, cs[q]:cs[q] + 128],
                                pt[:, cs[q]:cs[q] + 128],
                                msk01[:])
                    pend[k] = (pt, i0, offs, ws, cs)

                def emit_av(k):
                    p, h = pairs[k]
                    pt, i0, offs, ws, cs = pend.pop(k)
                    for q in range(2):
                        ii = i0 + q
                        nc.tensor.matmul(
                            av_ps[h][:, offs[q]:],
                            v_sb[h][:, 66 * ii:66 * ii + 65],
                            pt[:, cs[q]:cs[q] + ws[q]],
                            start=(ii == 0), stop=(ii == ndiag + 3))

                fl = list(filler)
                fi = 0
                n = len(pairs)
                for k in range(n + look):
                    if k < n:
                        emit_score(k)
                    want = min(len(fl), (len(fl) * (k + 1)) // max(1, n))
                    while fi < want:
                        fl[fi]()
                        fi += 1
                    if k >= look:
                        emit_av(k - look)
                while fi < len(fl):
                    fl[fi]()
                    fi += 1
                for h in range(2):
                    # softmax normalize: sums rode the AV matmul as row 64
                    rec = wp.tile([1, NQ], F32, name="rec")
                    nc.vector.reciprocal(rec[:], av_ps[h][64:65, :])
                    bc = wp.tile([64, NQ], F32, name="bc")
                    nc.gpsimd.partition_broadcast(bc[:], rec[:])
                    nc.vector.tensor_mul(
                        attnT[h][:, NQ * j:NQ * (j + 1)],
                        av_ps[h][0:64, :], bc[:])

            def a2a_stage(j=None):
                # A2A staging: shard attn^T along s (chunk j covers the
                # s-slices of dest cores 2j and 2j+1)
                # outbound writes ride the Activation HWDGE queue so inbound
                # loads (SP queue) never make compute wait on them
                rng = range(NCORES) if j is None else (2 * j, 2 * j + 1)
                for r in rng:
                    for h in range(2):
                        nc.scalar.dma_start(
                            a2a_in.ap()[r, 64 * h:64 * (h + 1), :],
                            attnT[h][:, SC * r:SC * (r + 1)])

            def wo_stage():
                src = a2a_out if collective else a2a_in
                for t in range(DT):
                    nc.sync.dma_start(at[t][:], src.ap()[t, :, :])
                for n in range(2):          # m-chunks of 512
                    y_ps = [ps.tile([128, 512], F32, name=f"y_ps{sub}",
                                    tag="mm") for sub in range(SC // 128)]
                    for t in range(DT):
                        for sub in range(SC // 128):
                            nc.tensor.matmul(
                                y_ps[sub][:],
                                at[t][:, 128 * sub:128 * (sub + 1)],
                                wot_sb[:, D * t + 512 * n:
                                       D * t + 512 * (n + 1)],
                                start=(t == 0), stop=(t == DT - 1))
                    for sub in range(SC // 128):
                        y_sb = wp.tile([128, 512], F32, name="y_sb")
                        nc.scalar.copy(y_sb[:], y_ps[sub][:])
                        nc.scalar.dma_start(
                            y_d[128 * sub:128 * (sub + 1),
                                512 * n:512 * (n + 1)], y_sb[:])

            pres = {0: [lambda: trig_load(2), lambda: xt_load(2),
                        lambda: wot_load(0)],
                    1: [lambda: trig_load(3), lambda: xt_load(3),
                        lambda: wot_load(1)],
                    2: [lambda: wot_load(2)],
                    3: [lambda: wot_load(3)]}

            def full_pass(loads=True):
                if loads:
                    head_loads()
                ones_load()
                if interleave:
                    for tok in qkv_tokens(0):
                        tok()
                    for j in range(NJ):
                        att_chunk(j, qkv_tokens(j + 1) if j + 1 < NJ else (),
                                  pres[j] if loads else ())
                        a2a_stage(j)
                else:
                    for j in range(NJ):
                        for tok in qkv_tokens(j):
                            tok()
                    for j in range(NJ):
                        att_chunk(j, (), pres[j] if loads else ())
                        a2a_stage(j)

            mode = loop_stages[0] if loop_stages else "single"
            if reps == 1:
                full_pass()
            elif mode == "unroll":
                head_loads()
                for jc in (2, 3):
                    trig_load(jc)
                    xt_load(jc)
                for q in range(4):
                    wot_load(q)
                for _ in range(reps):
                    full_pass(loads=False)
                    wo_stage()
            elif mode == "single":
                head_loads()
                for jc in (2, 3):
                    trig_load(jc)
                    xt_load(jc)
                for q in range(4):
                    wot_load(q)
                with tc.For_i(0, reps, 1, hint_engines=_HINTS):
                    full_pass(loads=False)
                    wo_stage()
            elif mode == "qk":
                head_loads()
                for jc in (2, 3):
                    trig_load(jc)
                    xt_load(jc)
                with tc.For_i(0, reps, 1, hint_engines=_HINTS):
                    ones_load()
                    for j in range(NJ):
                        for tok in qkv_tokens(j):
                            tok()
            elif mode == "att":
                head_loads()
                for jc in (2, 3):
                    trig_load(jc)
                    xt_load(jc)
                ones_load()
                for j in range(NJ):
                    for tok in qkv_tokens(j):
                        tok()
                with tc.For_i(0, reps, 1, hint_engines=_HINTS):
                    for j in range(NJ):
                        att_chunk(j)
                        a2a_stage(j)
            elif mode == "wo":
                head_loads()
                for q in range(4):
                    wot_load(q)
                with tc.For_i(0, reps, 1, hint_engines=_HINTS):
                    wo_stage()

            if collective:
                nc.gpsimd.collective_compute(
                    "AllToAll", ALU.bypass,
                    replica_groups=[list(range(NCORES))],
                    ins=[a2a_in.ap().opt()],
                    outs=[a2a_out.ap().opt()],
                )
            if reps == 1:
                wo_stage()
            if debug_taps:
                taps = {"qt_dbg": qt, "kt_dbg": kt,
                        "v0_dbg": v_sb[0], "v1_dbg": v_sb[1],
                        "at0_dbg": attnT[0], "at1_dbg": attnT[1]}
                for nm, tl in taps.items():
                    dbg = nc.dram_tensor(nm, list(tl.shape), F16,
                                         kind="ExternalOutput").ap()
                    nc.sync.dma_start(dbg[:, :], tl[:])

    nc.compile()
    return nc


def _get_program():
    global _PROGRAM
    if _PROGRAM is None:
        _PROGRAM = _build_program()
    return _PROGRAM


def _f16(a):
    return np.asarray(a, dtype=np.float16)


def _host_prep(x, token_positions, WQ, WK, WV, WO):
    x = np.asarray(x, dtype=np.float32)
    WQ = np.asarray(WQ, dtype=np.float32)
    WK = np.asarray(WK, dtype=np.float32)
    WV = np.asarray(WV, dtype=np.float32)
    WO = np.asarray(WO, dtype=np.float32)
    pos = np.asarray(token_positions).reshape(-1).astype(np.float32)

    xt = _f16(np.ascontiguousarray(x.reshape(S, D).T))     # [D, S]

    inv_freq = (1.0 / (THETA ** (np.arange(0, DK, 2, dtype=np.float32)
                                 / np.float32(DK)))).astype(np.float32)
    ang = pos[:, None] * inv_freq[None, :]                  # [S, 32] f32
    cos = np.cos(ang).astype(np.float32).T                  # [32, S]
    sin = np.sin(ang).astype(np.float32).T
    ctab = _f16(np.ascontiguousarray(np.tile(cos, (4, 1))))   # [128, S]
    stab = _f16(np.ascontiguousarray(
        np.concatenate([-sin, sin, -sin, sin], axis=0)))    # [128, S]

    pswap = np.zeros((128, 128), np.float32)
    for i in range(128):
        blk, o = divmod(i, 32)
        j = (blk ^ 1) * 32 + o
        pswap[j, i] = 1.0

    # mtri[k, q] = -60000 where q < k (dominates logits, fits fp16,
    # exp underflows to exactly 0) (upper triangle of S^T diag block)
    mtri = np.where(np.arange(128)[None, :] < np.arange(128)[:, None],
                    np.float32(-60000.0), np.float32(0.0))

    def pack(w):
        # [D, EC] -> SBUF layout [128, DT*EC]: cols t*EC+e = w[128t+p, e]
        return _f16(np.concatenate(
            [w[128 * t:128 * (t + 1), :] for t in range(DT)], axis=1))

    wot_pk = pack(np.ascontiguousarray(WO.T))               # [128, DT*D]

    perm = np.concatenate([np.arange(0, DK, 2), np.arange(1, DK, 2)])
    in_maps = []
    for c in range(NCORES):
        rows = np.concatenate([128 * c + 64 * l + perm for l in range(2)])
        wqt = np.ascontiguousarray(WQ[rows, :].T)           # [D, EC]
        wkt = np.ascontiguousarray(WK[rows, :].T)
        vrows = np.arange(128 * c, 128 * (c + 1))
        wvt = np.ascontiguousarray(WV[vrows, :].T)          # [D, EC]
        in_maps.append({
            "xt": xt, "wqt": pack(wqt), "wkt": pack(wkt),
            "wvt": pack(wvt), "wot": wot_pk,
            "ctab": ctab, "stab": stab, "pswap": _f16(pswap),
            "mtri": _f16(mtri), "ident": _f16(np.eye(128, dtype=np.float32)),
            "msk01": _f16((mtri == 0.0).astype(np.float32)),
            "ones": _f16(np.ones((128, KT), np.float32)),
        })
    return in_maps


def kernel(x, token_positions, WQ, WK, WV, WO):
    in_maps = _host_prep(x, token_positions, WQ, WK, WV, WO)
    nc = _get_program()
    res = run_bass_kernel_spmd(nc, in_maps, list(range(NCORES)))
    y = np.concatenate([res.results[c]["y_out"] for c in range(NCORES)],
                       axis=0)
    return y.reshape(1, S, D).astype(np.float32)

